# revision 7
# baseline (speedup 1.0000x reference)
"""VQ codebook (vector-quantization) kernel for Trainium2, 8-core data-parallel.

Problem: z [16,256,64,64] f32, codebook [1024,256] f32.
  zf = NCHW->NHWC flatten -> [65536, 256]
  d(n,k) = ||zf_n||^2 + ||e_k||^2 - 2 zf_n . e_k ; idx = argmin_k d
  zq = codebook[idx] (returned in NCHW), loss = 0.75 * mean((zq - z)^2)
  (the straight-through output equals zq in forward; the two loss terms share
   the same forward value, so loss = (1-0.25)*mean((zq-zc)^2)).

Numerics: the reference computes d in fp32 with a ~256 constant (||z||^2), so
d is quantized at ulp ~1.5e-5 and near-tie argmins are decided by that exact
rounding. We replicate the same op order bitwise where possible:
  A_n = fp32 sumsq of token, B_k = fp32 sumsq of code,
  s1 = fl(B + A)  (ACT affine add), C = 2*z.e via matmul against 2*codebook
  (scaling one operand by 2 is exact), d = fl(s1 - C) (DVE), argmin with
  first-occurrence tie-break via max_index.

Sharding: batch-parallel. Core i gets batches [2i, 2i+2) -> 8192 tokens/core;
codebook replicated.

Per-core pipeline (tokens tiled 128/partition):
  PE    : zT = transpose(z chunk); P2[tok,k] = z . (2e) fp32 matmul
  ACT   : A = Square+accum(zT); s1 = B_tile + A (per-partition bias)
  DVE   : tensor_tensor_reduce: d = s1 - P2 fused with running min -> m
  DVE   : max_index: first position of m in d -> idx (jnp argmin tie-break)
  GPSIMD: indirect DMA gather codebook[idx] -> [tok, 256]
  PE    : transpose gathered rows -> [c, tok] NCHW layout, ACT copies, DMA out
  loss  : sum(min d) == sum(z - zq)^2 exactly; host does the final scale.
"""

import numpy as np

B, C, HW, K = 16, 256, 4096, 1024
NCORES = 8
BPC = B // NCORES          # batches per core
NTOK = BPC * HW            # tokens per core
SUPER = 1024               # tokens per super-tile (DMA batching)
NSUP = NTOK // SUPER       # 8 super-tiles per core
SUBT = SUPER // 128        # 8 subtiles per super-tile
NTILES = NTOK // 128       # 64 token-tiles per core

_CACHE = {}


def _build_program():
    import concourse.bacc as bacc
    import concourse.bass as bass
    import concourse.mybir as mybir
    import concourse.tile as tile
    from concourse import bass_isa
    from concourse.masks import make_identity

    fp32 = mybir.dt.float32
    i32 = mybir.dt.int32
    u32 = mybir.dt.uint32
    AF = mybir.ActivationFunctionType
    OP = mybir.AluOpType

    nc = bacc.Bacc("TRN2", target_bir_lowering=False)

    z_d = nc.dram_tensor("z", [BPC, C, HW], fp32, kind="ExternalInput")
    cb_d = nc.dram_tensor("codebook", [K, C], fp32, kind="ExternalInput")
    zq_d = nc.dram_tensor("zq", [BPC, C, HW], fp32, kind="ExternalOutput")
    idx_d = nc.dram_tensor("idx", [NTILES, 128], i32, kind="ExternalOutput")
    stats_d = nc.dram_tensor("stats", [1, 1], fp32, kind="ExternalOutput")
    scr_d = nc.dram_tensor("scratch", [K], fp32, kind="Internal")

    with tile.TileContext(nc) as tc:
        with (
            tc.tile_pool(name="const", bufs=1) as cons,
            tc.tile_pool(name="zin", bufs=2) as zin,
            tc.tile_pool(name="wp", bufs=2) as wp,
            tc.tile_pool(name="gat", bufs=10) as gat,
            tc.tile_pool(name="zqt", bufs=2) as zqtp,
            tc.tile_pool(name="sq", bufs=2) as sqp,
            tc.tile_pool(name="small", bufs=4) as small,
            tc.tile_pool(name="pmm", bufs=2, space="PSUM") as pmm,
            tc.tile_pool(name="pzt", bufs=2, space="PSUM") as pzt,
            tc.tile_pool(name="ptr", bufs=2, space="PSUM") as ptr,
        ):
            # ---------------- setup ----------------
            ident = cons.tile([128, 128], fp32)
            make_identity(nc, ident[:])

            # codebook tiles [128, 256] x 8 (k-major)
            cb_tiles = []
            for j in range(8):
                t = cons.tile([128, C], fp32, tag=f"cb{j}", name=f"cb{j}")
                nc.sync.dma_start(out=t[:], in_=cb_d[j * 128:(j + 1) * 128, :])
                cb_tiles.append(t)

            # cbT2 chunks: [128 c, 1024 k] x 2 = transpose(codebook) * 2
            # (x2 is exact; makes the matmul produce 2*z.e bit-identically)
            cbT2 = [cons.tile([128, K], fp32, tag=f"cbT{h}", name=f"cbT{h}")
                    for h in range(2)]
            for j in range(8):
                for h in range(2):
                    pt = ptr.tile([128, 128], fp32, tag="pt", name="pt")
                    nc.tensor.transpose(
                        out=pt[:], in_=cb_tiles[j][:, h * 128:(h + 1) * 128],
                        identity=ident[:])
                    nc.scalar.activation(
                        out=cbT2[h][:, j * 128:(j + 1) * 128], in_=pt[:],
                        func=AF.Copy, scale=2.0)

            # B_k = ||e_k||^2 -> bias tile [128, 1024] (bcast over partitions)
            hn = cons.tile([128, 8], fp32, tag="hn")
            for j in range(8):
                sq = sqp.tile([128, C], fp32, tag="sqcb", name="sqcb")
                nc.scalar.activation(out=sq[:], in_=cb_tiles[j][:], func=AF.Square)
                nc.vector.tensor_reduce(out=hn[:, j:j + 1], in_=sq[:],
                                        axis=mybir.AxisListType.X, op=OP.add)
            hnT_ps = ptr.tile([128, 128], fp32, tag="pt", name="hnT_ps")
            nc.tensor.transpose(out=hnT_ps[:8, :], in_=hn[:, :8], identity=ident[:])
            hnT = cons.tile([8, 128], fp32, tag="hnT")
            nc.scalar.activation(out=hnT[:], in_=hnT_ps[:8, :], func=AF.Copy)
            nc.sync.dma_start(out=scr_d[:].rearrange("(a b) -> a b", b=128), in_=hnT[:])
            nrow = cons.tile([1, K], fp32, tag="nrow")
            nc.sync.dma_start(out=nrow[:], in_=scr_d[:].rearrange("(o a) -> o a", o=1))
            btile = cons.tile([128, K], fp32, tag="btile")
            nc.gpsimd.partition_broadcast(out_ap=btile[:], in_ap=nrow[:])

            # accumulator strips
            mstrip = cons.tile([128, NTILES], fp32, tag="mstrip")
            istrip = cons.tile([128, NTILES], fp32, tag="istrip")
            astrip = cons.tile([128, NTILES], fp32, tag="astrip")

            # ---------------- main loop ----------------
            for b in range(BPC):
                for st in range(HW // SUPER):
                    sup = b * (HW // SUPER) + st
                    t0 = st * SUPER
                    zc = []
                    for h in range(2):
                        t = zin.tile([128, SUPER], fp32, tag=f"z{h}", name=f"z{h}")
                        nc.sync.dma_start(
                            out=t[:], in_=z_d[b, h * 128:(h + 1) * 128, t0:t0 + SUPER])
                        zc.append(t)
                    gts = []
                    for sub in range(SUBT):
                        ti = sup * SUBT + sub
                        s0 = sub * 128
                        # A_n = ||z_n||^2 : transpose z chunk, square-accum on ACT
                        zt = pzt.tile([128, C], fp32, tag="zt", name="zt")
                        for h in range(2):
                            nc.tensor.transpose(
                                out=zt[:, h * 128:(h + 1) * 128],
                                in_=zc[h][:, s0:s0 + 128], identity=ident[:])
                        sqz = sqp.tile([128, C], fp32, tag="sqz", name="sqz")
                        nc.scalar.activation(
                            out=sqz[:], in_=zt[:], func=AF.Square,
                            accum_out=astrip[:, ti:ti + 1])
                        # P2 = 2 * z.e
                        ps = pmm.tile([128, K], fp32, tag="ps", name="ps")
                        for h in range(2):
                            nc.tensor.matmul(
                                out=ps[:, 0:512], lhsT=zc[h][:, s0:s0 + 128],
                                rhs=cbT2[h][:, 0:512], start=(h == 0), stop=(h == 1))
                            nc.tensor.matmul(
                                out=ps[:, 512:1024], lhsT=zc[h][:, s0:s0 + 128],
                                rhs=cbT2[h][:, 512:1024], start=(h == 0), stop=(h == 1))
                        # d = fl(fl(B + A) - P2)  (reference fp32 op order)
                        d = wp.tile([128, K], fp32, tag="d", name="d")
                        jacc = small.tile([128, 1], fp32, tag="jacc", name="jacc")
                        nc.vector.scalar_tensor_tensor(
                            out=d[:], in0=btile[:], scalar=astrip[:, ti:ti + 1],
                            in1=ps[:], op0=OP.add, op1=OP.subtract,
                            accum_out=jacc[:])
                        nc.vector.tensor_reduce(
                            out=mstrip[:, ti:ti + 1], in_=d[:],
                            axis=mybir.AxisListType.X, op=OP.min)
                        # first index of the min (jnp.argmin tie semantics)
                        m8 = small.tile([128, 8], fp32, tag="m8", name="m8")
                        nc.vector.tensor_copy(
                            out=m8[:], in_=mstrip[:, ti:ti + 1].to_broadcast([128, 8]))
                        idx8 = small.tile([128, 8], u32, tag="idx8", name="idx8")
                        nc.vector.max_index(out=idx8[:], in_max=m8[:], in_values=d[:])
                        nc.vector.tensor_copy(out=istrip[:, ti:ti + 1],
                                              in_=idx8[:, 0:1])
                        idx_i = small.tile([128, 1], i32, tag="idxi", name="idxi")
                        nc.vector.tensor_copy(out=idx_i[:], in_=idx8[:, 0:1])
                        # gather codebook rows
                        g = gat.tile([128, C], fp32, tag="g", name="g")
                        nc.gpsimd.indirect_dma_start(
                            out=g[:], out_offset=None, in_=cb_d[:],
                            in_offset=bass.IndirectOffsetOnAxis(ap=idx_i[:, :1], axis=0))
                        gts.append(g)
                    # pass B: transpose gathered rows into NCHW staging, write out
                    zqt = [zqtp.tile([128, SUPER], fp32, tag=f"zqt{h}", name=f"zqt{h}")
                           for h in range(2)]
                    for sub in range(SUBT):
                        s0 = sub * 128
                        for h in range(2):
                            pt = ptr.tile([128, 128], fp32, tag="pt", name="pt")
                            nc.tensor.transpose(
                                out=pt[:], in_=gts[sub][:, h * 128:(h + 1) * 128],
                                identity=ident[:])
                            nc.scalar.activation(
                                out=zqt[h][:, s0:s0 + 128], in_=pt[:], func=AF.Copy)
                    for h in range(2):
                        nc.sync.dma_start(
                            out=zq_d[b, h * 128:(h + 1) * 128, t0:t0 + SUPER],
                            in_=zqt[h][:])

            # ---------------- finalization ----------------
            # idx strip [128, 64] -> transpose -> [64, 128] int32 -> DRAM
            ips = ptr.tile([128, 128], fp32, tag="pt", name="ips")
            nc.tensor.transpose(out=ips[:NTILES, :], in_=istrip[:, :NTILES],
                                identity=ident[:])
            idx_sb = cons.tile([NTILES, 128], i32, tag="idxsb")
            nc.vector.tensor_copy(out=idx_sb[:], in_=ips[:NTILES, :])
            nc.sync.dma_start(out=idx_d[:], in_=idx_sb[:])

            # stats: sum of min-d == SSE
            red = cons.tile([128, 1], fp32, tag="red")
            nc.vector.tensor_reduce(out=red[:, 0:1], in_=mstrip[:],
                                    axis=mybir.AxisListType.X, op=OP.add)
            redall = cons.tile([128, 1], fp32, tag="redall")
            nc.gpsimd.partition_all_reduce(out_ap=redall[:], in_ap=red[:],
                                           channels=128,
                                           reduce_op=bass_isa.ReduceOp.add)
            nc.sync.dma_start(out=stats_d[:], in_=redall[0:1, :])

    nc.finalize()
    return nc


def _get_program():
    if "nc" not in _CACHE:
        _CACHE["nc"] = _build_program()
    return _CACHE["nc"]


def kernel(z: np.ndarray, codebook: np.ndarray):
    from concourse.bass_utils import run_bass_kernel_spmd

    z = np.ascontiguousarray(z, dtype=np.float32)
    codebook = np.ascontiguousarray(codebook, dtype=np.float32)

    nc = _get_program()
    in_maps = []
    for i in range(NCORES):
        zs = np.ascontiguousarray(
            z[i * BPC:(i + 1) * BPC].reshape(BPC, C, HW))
        in_maps.append({"z": zs, "codebook": codebook})

    res = run_bass_kernel_spmd(nc, in_maps, core_ids=list(range(NCORES)))
    outs = res.results

    zq = np.concatenate([o["zq"].reshape(BPC, C, 64, 64) for o in outs], axis=0)
    idx = np.concatenate([o["idx"].reshape(-1).astype(np.int32) for o in outs])
    sse = 0.0
    for o in outs:
        sse += float(o["stats"].reshape(1)[0])
    loss = np.float32(0.75 * sse / (B * C * HW))
    return zq, idx, loss


# revision 8
# speedup vs baseline: 12427.4639x; 12427.4639x over previous
"""VQ codebook (vector-quantization) kernel for Trainium2, 8-core data-parallel.

Problem: z [16,256,64,64] f32, codebook [1024,256] f32.
  zf = NCHW->NHWC flatten -> [65536, 256]
  d(n,k) = ||zf_n||^2 + ||e_k||^2 - 2 zf_n . e_k ; idx = argmin_k d
  zq = codebook[idx] (returned in NCHW), loss = 0.75 * mean((zq - z)^2)
  (the straight-through output equals zq in forward; the two loss terms share
   the same forward value, so loss = (1-0.25)*mean((zq-zc)^2)).

Numerics: the reference computes d in fp32 with a ~256 constant (||z||^2), so
d is quantized at ulp ~1.5e-5 and near-tie argmins are decided by that exact
rounding. We replicate the same op order bitwise where possible:
  A_n = fp32 sumsq of token, B_k = fp32 sumsq of code,
  s1 = fl(B + A)  (ACT affine add), C = 2*z.e via matmul against 2*codebook
  (scaling one operand by 2 is exact), d = fl(s1 - C) (DVE), argmin with
  first-occurrence tie-break via max_index.

Sharding: batch-parallel. Core i gets batches [2i, 2i+2) -> 8192 tokens/core;
codebook replicated.

Per-core pipeline (tokens tiled 128/partition):
  PE    : zT = transpose(z chunk); P2[tok,k] = z . (2e) fp32 matmul
  ACT   : A = Square+accum(zT); s1 = B_tile + A (per-partition bias)
  DVE   : tensor_tensor_reduce: d = s1 - P2 fused with running min -> m
  DVE   : max_index: first position of m in d -> idx (jnp argmin tie-break)
  GPSIMD: indirect DMA gather codebook[idx] -> [tok, 256]
  PE    : transpose gathered rows -> [c, tok] NCHW layout, ACT copies, DMA out
  loss  : sum(min d) == sum(z - zq)^2 exactly; host does the final scale.
"""

import numpy as np

B, C, HW, K = 16, 256, 4096, 1024
NCORES = 8
BPC = B // NCORES          # batches per core
NTOK = BPC * HW            # tokens per core
SUPER = 1024               # tokens per super-tile (DMA batching)
NSUP = NTOK // SUPER       # 8 super-tiles per core
SUBT = SUPER // 128        # 8 subtiles per super-tile
NTILES = NTOK // 128       # 64 token-tiles per core

_CACHE = {}


def _build_program():
    import concourse.bacc as bacc
    import concourse.bass as bass
    import concourse.mybir as mybir
    import concourse.tile as tile
    from concourse import bass_isa
    from concourse.masks import make_identity

    fp32 = mybir.dt.float32
    i32 = mybir.dt.int32
    u32 = mybir.dt.uint32
    AF = mybir.ActivationFunctionType
    OP = mybir.AluOpType

    nc = bacc.Bacc("TRN2", target_bir_lowering=False)

    z_d = nc.dram_tensor("z", [BPC, C, HW], fp32, kind="ExternalInput")
    cb_d = nc.dram_tensor("codebook", [K, C], fp32, kind="ExternalInput")
    zq_d = nc.dram_tensor("zq", [BPC, C, HW], fp32, kind="ExternalOutput")
    idx_d = nc.dram_tensor("idx", [NTILES, 128], i32, kind="ExternalOutput")
    stats_d = nc.dram_tensor("stats", [1, 1], fp32, kind="ExternalOutput")
    scr_d = nc.dram_tensor("scratch", [K], fp32, kind="Internal")

    with tile.TileContext(nc) as tc:
        with (
            tc.tile_pool(name="const", bufs=1) as cons,
            tc.tile_pool(name="zin", bufs=2) as zin,
            tc.tile_pool(name="wp", bufs=2) as wp,
            tc.tile_pool(name="gat", bufs=10) as gat,
            tc.tile_pool(name="zqt", bufs=2) as zqtp,
            tc.tile_pool(name="sq", bufs=2) as sqp,
            tc.tile_pool(name="small", bufs=4) as small,
            tc.tile_pool(name="pmm", bufs=2, space="PSUM") as pmm,
            tc.tile_pool(name="pzt", bufs=2, space="PSUM") as pzt,
            tc.tile_pool(name="ptr", bufs=2, space="PSUM") as ptr,
        ):
            # ---------------- setup ----------------
            ident = cons.tile([128, 128], fp32)
            make_identity(nc, ident[:])

            # codebook tiles [128, 256] x 8 (k-major)
            cb_tiles = []
            for j in range(8):
                t = cons.tile([128, C], fp32, tag=f"cb{j}", name=f"cb{j}")
                nc.sync.dma_start(out=t[:], in_=cb_d[j * 128:(j + 1) * 128, :])
                cb_tiles.append(t)

            # cbT2 chunks: [128 c, 1024 k] x 2 = transpose(codebook) * 2
            # (x2 is exact; makes the matmul produce 2*z.e bit-identically)
            cbT2 = [cons.tile([128, K], fp32, tag=f"cbT{h}", name=f"cbT{h}")
                    for h in range(2)]
            for j in range(8):
                for h in range(2):
                    pt = ptr.tile([128, 128], fp32, tag="pt", name="pt")
                    nc.tensor.transpose(
                        out=pt[:], in_=cb_tiles[j][:, h * 128:(h + 1) * 128],
                        identity=ident[:])
                    nc.scalar.activation(
                        out=cbT2[h][:, j * 128:(j + 1) * 128], in_=pt[:],
                        func=AF.Copy, scale=2.0)

            # B_k = ||e_k||^2 -> bias tile [128, 1024] (bcast over partitions)
            hn = cons.tile([128, 8], fp32, tag="hn")
            for j in range(8):
                sq = sqp.tile([128, C], fp32, tag="sqcb", name="sqcb")
                nc.scalar.activation(out=sq[:], in_=cb_tiles[j][:], func=AF.Square)
                nc.vector.tensor_reduce(out=hn[:, j:j + 1], in_=sq[:],
                                        axis=mybir.AxisListType.X, op=OP.add)
            hnT_ps = ptr.tile([128, 128], fp32, tag="pt", name="hnT_ps")
            nc.tensor.transpose(out=hnT_ps[:8, :], in_=hn[:, :8], identity=ident[:])
            hnT = cons.tile([8, 128], fp32, tag="hnT")
            nc.scalar.activation(out=hnT[:], in_=hnT_ps[:8, :], func=AF.Copy)
            nc.sync.dma_start(out=scr_d[:].rearrange("(a b) -> a b", b=128), in_=hnT[:])
            nrow = cons.tile([1, K], fp32, tag="nrow")
            nc.sync.dma_start(out=nrow[:], in_=scr_d[:].rearrange("(o a) -> o a", o=1))
            btile = cons.tile([128, K], fp32, tag="btile")
            nc.gpsimd.partition_broadcast(out_ap=btile[:], in_ap=nrow[:])

            # accumulator strips
            mstrip = cons.tile([128, NTILES], fp32, tag="mstrip")
            istrip_i = cons.tile([128, NTILES], i32, tag="istrip_i")
            istrip = cons.tile([128, NTILES], fp32, tag="istrip")
            astrip = cons.tile([128, NTILES], fp32, tag="astrip")

            # ---------------- main loop ----------------
            for b in range(BPC):
                for st in range(HW // SUPER):
                    sup = b * (HW // SUPER) + st
                    t0 = st * SUPER
                    zc = []
                    for h in range(2):
                        t = zin.tile([128, SUPER], fp32, tag=f"z{h}", name=f"z{h}")
                        nc.sync.dma_start(
                            out=t[:], in_=z_d[b, h * 128:(h + 1) * 128, t0:t0 + SUPER])
                        zc.append(t)
                    gts = []
                    for sub in range(SUBT):
                        ti = sup * SUBT + sub
                        s0 = sub * 128
                        # A_n = ||z_n||^2 : transpose z chunk, square-accum on ACT
                        zt = pzt.tile([128, C], fp32, tag="zt", name="zt")
                        for h in range(2):
                            nc.tensor.transpose(
                                out=zt[:, h * 128:(h + 1) * 128],
                                in_=zc[h][:, s0:s0 + 128], identity=ident[:])
                        sqz = sqp.tile([128, C], fp32, tag="sqz", name="sqz")
                        nc.scalar.activation(
                            out=sqz[:], in_=zt[:], func=AF.Square,
                            accum_out=astrip[:, ti:ti + 1])
                        # P2 = 2 * z.e
                        ps = pmm.tile([128, K], fp32, tag="ps", name="ps")
                        for h in range(2):
                            nc.tensor.matmul(
                                out=ps[:, 0:512], lhsT=zc[h][:, s0:s0 + 128],
                                rhs=cbT2[h][:, 0:512], start=(h == 0), stop=(h == 1))
                            nc.tensor.matmul(
                                out=ps[:, 512:1024], lhsT=zc[h][:, s0:s0 + 128],
                                rhs=cbT2[h][:, 512:1024], start=(h == 0), stop=(h == 1))
                        # d = fl(fl(B + A) - P2)  (reference fp32 op order)
                        d = wp.tile([128, K], fp32, tag="d", name="d")
                        jacc = small.tile([128, 1], fp32, tag="jacc", name="jacc")
                        nc.vector.scalar_tensor_tensor(
                            out=d[:], in0=btile[:], scalar=astrip[:, ti:ti + 1],
                            in1=ps[:], op0=OP.add, op1=OP.subtract,
                            accum_out=jacc[:])
                        nc.vector.tensor_reduce(
                            out=mstrip[:, ti:ti + 1], in_=d[:],
                            axis=mybir.AxisListType.X, op=OP.min)
                        # first index of the min (jnp.argmin tie semantics)
                        m8 = small.tile([128, 8], fp32, tag="m8", name="m8")
                        nc.vector.tensor_copy(
                            out=m8[:], in_=mstrip[:, ti:ti + 1].to_broadcast([128, 8]))
                        idx8 = small.tile([128, 8], u32, tag="idx8", name="idx8")
                        nc.vector.max_index(out=idx8[:], in_max=m8[:], in_values=d[:])
                        nc.vector.tensor_copy(out=istrip_i[:, ti:ti + 1],
                                              in_=idx8[:, 0:1])
                        # gather codebook rows
                        g = gat.tile([128, C], fp32, tag="g", name="g")
                        nc.gpsimd.indirect_dma_start(
                            out=g[:], out_offset=None, in_=cb_d[:],
                            in_offset=bass.IndirectOffsetOnAxis(
                                ap=istrip_i[:, ti:ti + 1], axis=0))
                        gts.append(g)
                    # pass B: transpose gathered rows into NCHW staging, write out
                    zqt = [zqtp.tile([128, SUPER], fp32, tag=f"zqt{h}", name=f"zqt{h}")
                           for h in range(2)]
                    for sub in range(SUBT):
                        s0 = sub * 128
                        for h in range(2):
                            pt = ptr.tile([128, 128], fp32, tag="pt", name="pt")
                            nc.tensor.transpose(
                                out=pt[:], in_=gts[sub][:, h * 128:(h + 1) * 128],
                                identity=ident[:])
                            nc.scalar.activation(
                                out=zqt[h][:, s0:s0 + 128], in_=pt[:], func=AF.Copy)
                    for h in range(2):
                        ste = zqtp.tile([128, SUPER], fp32, tag=f"ste{h}",
                                        name=f"ste{h}")
                        nc.gpsimd.tensor_tensor(
                            out=ste[:], in0=zqt[h][:], in1=zc[h][:],
                            op=OP.subtract)
                        nc.gpsimd.tensor_add(out=ste[:], in0=zc[h][:], in1=ste[:])
                        nc.sync.dma_start(
                            out=zq_d[b, h * 128:(h + 1) * 128, t0:t0 + SUPER],
                            in_=ste[:])

            # ---------------- finalization ----------------
            # idx strip [128, 64] -> transpose -> [64, 128] int32 -> DRAM
            nc.vector.tensor_copy(out=istrip[:], in_=istrip_i[:])
            ips = ptr.tile([128, 128], fp32, tag="pt", name="ips")
            nc.tensor.transpose(out=ips[:NTILES, :], in_=istrip[:, :NTILES],
                                identity=ident[:])
            idx_sb = cons.tile([NTILES, 128], i32, tag="idxsb")
            nc.vector.tensor_copy(out=idx_sb[:], in_=ips[:NTILES, :])
            nc.sync.dma_start(out=idx_d[:], in_=idx_sb[:])

            # stats: sum of min-d == SSE
            red = cons.tile([128, 1], fp32, tag="red")
            nc.vector.tensor_reduce(out=red[:, 0:1], in_=mstrip[:],
                                    axis=mybir.AxisListType.X, op=OP.add)
            redall = cons.tile([128, 1], fp32, tag="redall")
            nc.gpsimd.partition_all_reduce(out_ap=redall[:], in_ap=red[:],
                                           channels=128,
                                           reduce_op=bass_isa.ReduceOp.add)
            nc.sync.dma_start(out=stats_d[:], in_=redall[0:1, :])

    nc.finalize()
    return nc


def _get_program():
    if "nc" not in _CACHE:
        _CACHE["nc"] = _build_program()
    return _CACHE["nc"]


def kernel(z: np.ndarray, codebook: np.ndarray):
    from concourse.bass_utils import run_bass_kernel_spmd

    z = np.ascontiguousarray(z, dtype=np.float32)
    codebook = np.ascontiguousarray(codebook, dtype=np.float32)

    nc = _get_program()
    in_maps = []
    for i in range(NCORES):
        zs = np.ascontiguousarray(
            z[i * BPC:(i + 1) * BPC].reshape(BPC, C, HW))
        in_maps.append({"z": zs, "codebook": codebook})

    res = run_bass_kernel_spmd(nc, in_maps, core_ids=list(range(NCORES)))
    outs = res.results

    zq = np.concatenate([o["zq"].reshape(BPC, C, 64, 64) for o in outs], axis=0)
    idx = np.concatenate([o["idx"].reshape(-1).astype(np.int32) for o in outs])
    sse = 0.0
    for o in outs:
        sse += float(o["stats"].reshape(1)[0])
    loss = np.float32(0.75 * sse / (B * C * HW))
    return zq, idx, loss
